# revision 1
# baseline (speedup 1.0000x reference)
"""Multi-head causal attention with RoPE on 8 Trainium2 NeuronCores.

Sharding: tensor-parallel over heads — core c owns heads (2c, 2c+1) for both
batch rows. QKV projection computed column-sliced per core; attention is fully
local per head; the output projection is row-parallel (each core computes a
full-shape partial product) and the 8 partials are summed on the host.

On-device layout is fully "transposed": q,k live as [head_dim, T] so attention
scores are built as S^T[k,q] tiles, softmax normalization lands per-column,
and the PV matmul consumes exp(S^T) directly with v in natural [T, head_dim]
layout — no transposes anywhere in the hot loop. Matmuls run in float32r
(~13-bit mantissa, 4x the fp32 rate).
"""
import sys

sys.path.insert(0, "/opt/trn_rl_repo")

import numpy as np

B, T, D, H, HD = 2, 2048, 1024, 16, 64
NCORES = 8
HPC = H // NCORES  # heads per core = 2
QT = 512  # q-tile width (S^T free dim)
KT = 128  # k-tile width (S^T partition dim)
NQT = T // QT  # 4
NKT = T // KT  # 16
DT = 128  # d-chunk (contraction tiles)
NDT = D // DT  # 8

# flip these if a compile/verify experiment fails
BCAST_VIA_DMA = False  # 0-stride APs rejected by both DVE and DMA

_CACHE = {}


def _build():
    import concourse.bass as bass  # noqa: F401
    from concourse import bacc
    import concourse.mybir as mybir
    from concourse.tile import TileContext

    F32 = mybir.dt.float32
    F32R = mybir.dt.float32r
    AF = mybir.ActivationFunctionType

    nc = bacc.Bacc("TRN2", target_bir_lowering=False)

    XT = nc.dram_tensor("xt", [B, D, T], F32R, kind="ExternalInput")
    WQK = nc.dram_tensor("wqk", [D, 256], F32R, kind="ExternalInput")
    WV = nc.dram_tensor("wv", [D, 128], F32R, kind="ExternalInput")
    WOUT = nc.dram_tensor("wout", [128, D], F32R, kind="ExternalInput")
    F16 = mybir.dt.float16
    COS2 = nc.dram_tensor("cos2", [128, T], F16, kind="ExternalInput")
    SIN2 = nc.dram_tensor("sin2", [128, T], F16, kind="ExternalInput")
    P2T = nc.dram_tensor("p2t", [128, 128], F32R, kind="ExternalInput")
    TRIMASK = nc.dram_tensor("trimask", [128, 128], F32, kind="ExternalInput")
    IDENT = nc.dram_tensor("ident", [128, 128], F32R, kind="ExternalInput")
    if not BCAST_VIA_DMA:
        ONESBC = nc.dram_tensor("onesbc", [1, 64], F32R, kind="ExternalInput")
    ONESV = nc.dram_tensor("onesv", [128, 1], F32, kind="ExternalInput")
    OUTP = nc.dram_tensor("outp", [B, T, D], F32, kind="ExternalOutput")

    with TileContext(nc) as tc:
        with (
            tc.tile_pool(name="const", bufs=1) as cst,
            tc.tile_pool(name="xt", bufs=1) as xtp,
            tc.tile_pool(name="qk", bufs=3) as qkp,
            tc.tile_pool(name="vt", bufs=5) as vtp,
            tc.tile_pool(name="rtmp", bufs=2) as rtp,
            tc.tile_pool(name="v", bufs=24) as vp,
            tc.tile_pool(name="pt", bufs=5) as ptp,
            tc.tile_pool(name="ot", bufs=2) as otp,
            tc.tile_pool(name="sm", bufs=4) as smp,
            tc.tile_pool(name="ostage", bufs=3) as osp,
            tc.tile_pool(name="psA", bufs=4, space="PSUM") as psA,
            tc.tile_pool(name="psO", bufs=2, space="PSUM") as psO,
            tc.tile_pool(name="psVB", bufs=2, space="PSUM") as psVB,
        ):
            # ---- constants ----
            cos2 = cst.tile([128, T], F16, tag="cos2")
            sin2 = cst.tile([128, T], F16, tag="sin2")
            p2t = cst.tile([128, 128], F32R, tag="p2t")
            trimask = cst.tile([128, 128], F32, tag="trimask")
            ident = cst.tile([128, 128], F32R, tag="ident")
            if not BCAST_VIA_DMA:
                onesbc = cst.tile([1, 64], F32R, tag="onesbc")
            onesv = cst.tile([128, 1], F32, tag="onesv")
            wqk = []
            wv = []
            for d in range(NDT):
                t_ = cst.tile([DT, 256], F32R, tag=f"wqk{d}", name=f"wqk{d}")
                nc.sync.dma_start(t_[:], WQK[d * DT : (d + 1) * DT, :])
                wqk.append(t_[:])
                t_ = cst.tile([DT, 128], F32R, tag=f"wv{d}", name=f"wv{d}")
                wv.append(t_[:])
            wout = cst.tile([128, D], F32R, tag="wout")

            def load_late_consts():
                for d in range(NDT):
                    nc.sync.dma_start(wv[d], WV[d * DT : (d + 1) * DT, :])
                nc.sync.dma_start(ident[:], IDENT[:])
                nc.sync.dma_start(p2t[:], P2T[:])
                nc.sync.dma_start(cos2[:], COS2[:])
                nc.sync.dma_start(sin2[:], SIN2[:])
                nc.sync.dma_start(trimask[:], TRIMASK[:])
                if not BCAST_VIA_DMA:
                    nc.sync.dma_start(onesbc[:], ONESBC[:])
                nc.sync.dma_start(onesv[:], ONESV[:])
                nc.sync.dma_start(wout[:], WOUT[:])

            for b in range(B):
                # ---- QKV projection (batch b) ----
                with nc.named_scope(f"qkv{b}"):
                    # xt as [d, tj] chunks: fine-grained DMA so the first
                    # projection chains unblock early
                    xt = [[None] * NQT for _ in range(NDT)]
                    for tj in range(NQT):
                        for d in range(NDT):
                            t_ = xtp.tile(
                                [DT, QT], F32R, tag=f"xt{d}_{tj}", name=f"xt{d}_{tj}"
                            )
                            nc.sync.dma_start(
                                t_[:],
                                XT[b, d * DT : (d + 1) * DT, tj * QT : (tj + 1) * QT],
                            )
                            xt[d][tj] = t_
                    if b == 0:
                        load_late_consts()
                    # q,k transposed: [128(2 heads x 64), T]
                    qkt = {}
                    for g, name in ((0, "q"), (1, "k")):
                        dst = qkp.tile([128, T], F32R, tag="qkraw")
                        for tj in range(NQT):
                            ps = psA.tile([128, QT], F32, tag="big")
                            for d in range(NDT):
                                nc.tensor.matmul(
                                    ps[:],
                                    wqk[d][:, 128 * g : 128 * g + 128],
                                    xt[d][tj][:],
                                    start=(d == 0),
                                    stop=(d == NDT - 1),
                                )
                            nc.vector.tensor_copy(
                                dst[:, tj * QT : (tj + 1) * QT], ps[:]
                            )
                        qkt[name] = dst
                    # v: compute transposed [128 (v cols, 2 heads), T] with
                    # 512-wide matmuls, then PE-transpose into natural layout
                    vtr = []
                    for tj in range(NQT):
                        ps = psA.tile([128, QT], F32, tag="big")
                        for d in range(NDT):
                            nc.tensor.matmul(
                                ps[:],
                                wv[d],
                                xt[d][tj][:],
                                start=(d == 0),
                                stop=(d == NDT - 1),
                            )
                        vq = vtp.tile([128, QT], F32R, tag="vtr", name=f"vtr{tj}")
                        nc.scalar.copy(vq[:], ps[:])
                        vtr.append(vq)
                    vt = []
                    for ti in range(NKT):
                        psT = psVB.tile([128, 128], F32R, tag="vb", name="psT")
                        nc.tensor.transpose(
                            psT[:], vtr[ti // 4][:, (ti % 4) * KT : (ti % 4 + 1) * KT], ident[:]
                        )
                        v_ = vp.tile([128, 130], F32R, tag="v")
                        nc.scalar.copy(v_[:, 0:64], psT[:, 0:64])
                        nc.scalar.copy(v_[:, 65:129], psT[:, 64:128])
                        nc.vector.tensor_copy(v_[:, 64:65], onesv[:])
                        nc.vector.tensor_copy(v_[:, 129:130], onesv[:])
                        vt.append(v_)

                # ---- RoPE on q and k (in place, both heads at once) ----
                with nc.named_scope(f"rope{b}"):
                    for name in ("q", "k"):
                        raw = qkt[name]
                        for s in range(NQT):
                            sl = slice(s * QT, (s + 1) * QT)
                            psr = psA.tile([128, QT], F32, tag="big")
                            nc.tensor.matmul(
                                psr[:], p2t[:], raw[:, sl], start=True, stop=True
                            )
                            t1 = rtp.tile([128, QT], F32, tag="t1")
                            nc.vector.tensor_mul(t1[:], psr[:], sin2[:, sl])
                            t2 = rtp.tile([128, QT], F32, tag="t2")
                            nc.vector.tensor_mul(t2[:], raw[:, sl], cos2[:, sl])
                            nc.vector.tensor_add(raw[:, sl], t1[:], t2[:])

                # ---- attention per head ----
                ot = otp.tile([128, T], F32R, tag="ot")
                for j in range(NQT):
                    nk = 4 * j + 4
                    with nc.named_scope(f"attn{b}j{j}"):
                        pso = [psO.tile([65, QT], F32, tag="o", name=f"pso{_h}") for _h in range(HPC)]
                        for i in range(nk):
                            r = i - 4 * j  # >=0 only for diagonal tiles
                            off = 128 * r if r >= 0 else 0
                            pss_l = []
                            pt_l = []
                            for h in range(HPC):
                                hs = slice(64 * h, 64 * h + 64)
                                pss = psA.tile(
                                    [128, QT], F32, tag="big", name=f"pss{h}"
                                )
                                nc.tensor.matmul(
                                    pss[:, off:QT],
                                    qkt["k"][hs, i * KT : (i + 1) * KT],
                                    qkt["q"][hs, j * QT + off : (j + 1) * QT],
                                    start=True,
                                    stop=True,
                                )
                                pss_l.append(pss)
                            for h in range(HPC):
                                pt = ptp.tile([128, QT], F32R, tag="pt", name=f"pt{h}")
                                nc.scalar.activation(
                                    pt[:, off:QT],
                                    pss_l[h][:, off:QT],
                                    AF.Exp,
                                    scale=0.125,
                                )
                                if r >= 0:
                                    nc.vector.tensor_mul(
                                        pt[:, off : off + 128],
                                        pt[:, off : off + 128],
                                        trimask[:],
                                    )
                                pt_l.append(pt)
                            for h in range(HPC):
                                nc.tensor.matmul(
                                    pso[h][:, off:QT],
                                    vt[i][:, 65 * h : 65 * h + 65],
                                    pt_l[h][:, off:QT],
                                    start=(i == 0),
                                    stop=(i == nk - 1),
                                )
                        for h in range(HPC):
                            hs = slice(64 * h, 64 * h + 64)
                            rcr = smp.tile([1, QT], F32R, tag="rcr")
                            nc.scalar.copy(rcr[:], pso[h][64:65, :])
                            psb = psVB.tile([128, QT], F32, tag="vb", name="psb")
                            nc.tensor.matmul(
                                psb[0:64, :], onesbc[:], rcr[:], start=True, stop=True
                            )
                            rec = smp.tile([64, QT], F32, tag="rec")
                            nc.vector.reciprocal_approx_fast(rec[:], psb[0:64, :])
                            nc.vector.tensor_mul(
                                ot[hs, j * QT : (j + 1) * QT],
                                pso[h][0:64, :],
                                rec[:],
                            )

                # ---- output projection (row-parallel partial) ----
                with nc.named_scope(f"oproj{b}"):
                    for ti in range(NKT):
                        st = osp.tile([128, D], F32, tag="ost")
                        for g in range(2):
                            ps = psA.tile([128, QT], F32, tag="big")
                            nc.tensor.matmul(
                                ps[:],
                                ot[:, ti * KT : (ti + 1) * KT],
                                wout[:, g * QT : (g + 1) * QT],
                                start=True,
                                stop=True,
                            )
                            nc.vector.tensor_copy(
                                st[:, g * QT : (g + 1) * QT], ps[:]
                            )
                        nc.sync.dma_start(OUTP[b, ti * KT : (ti + 1) * KT, :], st[:])

    nc.compile()
    return nc


def _host_consts():
    pos = np.arange(T, dtype=np.float64)
    theta = 1.0 / (10000.0 ** (np.arange(0, HD, 2, dtype=np.float64) / HD))
    ang = pos[:, None] * theta[None, :]  # [T, 32]
    cos = np.tile(np.cos(ang), (1, 2)).T.astype(np.float16)  # [64, T]
    sin = np.tile(np.sin(ang), (1, 2)).T.astype(np.float16)
    cos2 = np.vstack([cos, cos])  # [128, T] two heads stacked
    sin2 = np.vstack([sin, sin])
    # rotate-half as a matmul: rot = P @ q for q in [64, t] column layout
    P = np.zeros((HD, HD), dtype=np.float32)
    for i_ in range(32):
        P[i_, i_ + 32] = -1.0
        P[i_ + 32, i_] = 1.0
    P2 = np.zeros((128, 128), dtype=np.float32)
    P2[0:64, 0:64] = P
    P2[64:128, 64:128] = P
    p2t = np.ascontiguousarray(P2.T)
    f, p = np.meshgrid(np.arange(128), np.arange(128))
    trimask = (p <= f).astype(np.float32)  # [p, f] valid iff p <= f
    ident = np.eye(128, dtype=np.float32)
    onesbc = np.ones((1, 64), dtype=np.float32)
    onesv = np.ones((128, 1), dtype=np.float32)
    return cos2, sin2, p2t, trimask, ident, onesbc, onesv


def kernel(x, w_qkv, w_out, b_out):
    from concourse.bass_utils import run_bass_kernel_spmd

    if "nc" not in _CACHE:
        _CACHE["nc"] = _build()
    nc = _CACHE["nc"]

    x = np.asarray(x, dtype=np.float32)
    w_qkv = np.asarray(w_qkv, dtype=np.float32)
    w_out = np.asarray(w_out, dtype=np.float32)
    b_out = np.asarray(b_out, dtype=np.float32)

    xt = np.ascontiguousarray(x.transpose(0, 2, 1))  # [B, D, T]
    cos2, sin2, p2t, trimask, ident, onesbc, onesv = _host_consts()

    wq = w_qkv[:, 0:D]
    wk = w_qkv[:, D : 2 * D]
    wv_full = w_qkv[:, 2 * D : 3 * D]

    in_maps = []
    for c in range(NCORES):
        h0, h1 = HPC * c, HPC * c + 1
        cols = []
        for w in (wq, wk):
            cols.append(w[:, h0 * HD : (h0 + 1) * HD])
            cols.append(w[:, h1 * HD : (h1 + 1) * HD])
        wqk_c = np.ascontiguousarray(np.concatenate(cols, axis=1))  # [D, 256]
        wv_c = np.ascontiguousarray(
            np.concatenate(
                [
                    wv_full[:, h0 * HD : (h0 + 1) * HD],
                    wv_full[:, h1 * HD : (h1 + 1) * HD],
                ],
                axis=1,
            )
        )  # [D, 128]
        wout_c = np.ascontiguousarray(
            np.concatenate(
                [
                    w_out[h0 * HD : (h0 + 1) * HD, :],
                    w_out[h1 * HD : (h1 + 1) * HD, :],
                ],
                axis=0,
            )
        )  # [128, D]
        in_maps.append(
            {
                "xt": xt,
                "wqk": wqk_c,
                "wv": wv_c,
                "wout": wout_c,
                "cos2": cos2,
                "sin2": sin2,
                "p2t": p2t,
                "trimask": trimask,
                "ident": ident,
                "onesbc": onesbc,
                "onesv": onesv,
            }
        )

    global _last_in_maps
    _last_in_maps = in_maps
    res = run_bass_kernel_spmd(nc, in_maps, list(range(NCORES)))
    acc = np.zeros((B, T, D), dtype=np.float64)
    for c in range(NCORES):
        acc += res.results[c]["outp"].astype(np.float64)
    acc += b_out.astype(np.float64)
    return acc.astype(np.float32)

